# revision 6
# baseline (speedup 1.0000x reference)
"""Trainium2 Bass kernel for nn_NormConvTranspose2d (v2, minimal device program).

Math: the reference applies, per (out-channel o, in-channel c), a
ConvTranspose2d(stride=2, k=3, pad=1, outpad=1) to input channel c with
kernel K[o,c], divides by the same convT applied to an all-ones image
(+eps), multiplies by weight[o,c], sums over c, adds bias.

With stride 2 / k 3, each output-pixel parity class (h'=2r+a, w'=2q+b)
is a fixed 1-4 tap correlation of the input, and the norm denominator is
a per-(o,c) constant within each parity class (except at the h'=95 /
w'=95 edges).  y/norm therefore folds into effective channel-mixing
matrices W_tap = weight*ktap/denom computed on the host, and the module
becomes channel-mixing matmuls over (shifted) input.

Device program (per core) computes ONLY the interior of the four parity
planes with 9 matmuls; everything else (plane interleave, h'=95 / w'=95
edge columns, bias) is cheap host post-processing:

  T1 = [x ; x<<1elem]  (128 partitions, built host-side in DRAM)
  P1 = [[Wee,Wf],[0,Wd]]  @ T1          -> [ee | eo]   (1 matmul)
  P2 = [[Wh,Wi],[0,Wg]]   @ T1          -> [oe | oo]   (accumulating
     + [[Wb,Wc],[0,Wa]]   @ (T1 << 48)                  pair)

Sharding: 8 cores = 4 batches x 2 output-row halves, no communication.
Each core: 2 input DMAs (bf16), 9 matmuls over 3 column-chunks
{512,512,128}, DVE/ACT psum->sbuf bf16 copies, 3 output DMAs.  A couple
of zero warmup matmuls run in the input-DMA shadow to ramp the PE
p-state.
"""

import numpy as np
import ml_dtypes

BF16 = ml_dtypes.bfloat16
EPS = 1e-10
B, C, O, H, W = 4, 64, 64, 48, 48
HO = WO = 96
SLAB = 25          # input rows per core (24 + halo)
L = SLAB * 48      # 1200
LP = 1216          # padded free size of x tile
CHUNKS = [(0, 512), (512, 512), (1024, 128)]
N_WARMUP = 7
WM = 384           # weight-map columns, stored ahead of x in the fused tile
XW = WM + LP       # 1600 total columns
SPLIT = 992        # first input-DMA piece [0:SPLIT) covers wm + chunk-0 x

_prog_cache = {}


def _build_program():
    import concourse.mybir as mybir
    import concourse.tile as tile
    from concourse import bacc

    f32 = mybir.dt.float32
    bf16 = mybir.dt.bfloat16
    Ident = mybir.ActivationFunctionType.Identity

    nc = bacc.Bacc("TRN2", target_bir_lowering=False, debug=False, num_devices=8)
    x_d = nc.dram_tensor("x", [128, LP], bf16, kind="ExternalInput").ap()
    w_d = nc.dram_tensor("wm", [128, 384], bf16, kind="ExternalInput").ap()
    out_d = nc.dram_tensor("out", [128, 2304], bf16, kind="ExternalOutput").ap()

    with tile.TileContext(nc) as tc:
        with (
            tc.tile_pool(name="const", bufs=1) as cpool,
            tc.tile_pool(name="outp", bufs=3) as opool,
            tc.tile_pool(name="psum", bufs=3, space="PSUM") as ppool,
            tc.tile_pool(name="psumw", bufs=1, space="PSUM") as wpool,
        ):
            # warm the Scalar activation table before any data arrives
            warm = cpool.tile([64, 1], f32)
            nc.vector.memset(warm[:], 0.0)
            nc.scalar.activation(warm[:], warm[:], Ident, bias=0.0)

            t1 = cpool.tile([128, LP], bf16)
            wm = cpool.tile([128, 384], bf16)
            nc.scalar.dma_start(wm[:], w_d[:])
            nc.sync.dma_start(t1[0:64, :], x_d[0:64, :])
            nc.gpsimd.dma_start(t1[64:128, :], x_d[64:128, :])

            # PE p-state warmup on zeros while the input DMA is in flight
            scr = cpool.tile([128, 512], bf16)
            nc.vector.memset(scr[:], 0.0)
            wps = wpool.tile([128, 512], f32, tag="wu")
            for _ in range(N_WARMUP):
                nc.tensor.matmul(wps[:], scr[:, 0:128], scr[:],
                                 start=True, stop=True)

            out_dmas = [nc.sync, nc.scalar, nc.sync]
            off = 0
            for ci, (fb, n) in enumerate(CHUNKS):
                p1 = ppool.tile([128, 512], f32, tag="p1")
                nc.tensor.matmul(p1[:, 0:n], wm[:, 0:128], t1[:, fb : fb + n],
                                 start=True, stop=True)
                p2 = ppool.tile([128, 512], f32, tag="p2")
                nc.tensor.matmul(p2[:, 0:n], wm[:, 128:256],
                                 t1[:, fb : fb + n], start=True, stop=False)
                nc.tensor.matmul(p2[:, 0:n], wm[:, 256:384],
                                 t1[:, fb + 48 : fb + 48 + n],
                                 start=False, stop=True)

                och = opool.tile([128, 1024], bf16)
                nc.vector.tensor_copy(och[:, 0:n], p1[:, 0:n])
                nc.scalar.copy(och[:, n : 2 * n], p2[:, 0:n])
                out_dmas[ci].dma_start(out_d[:, off : off + 2 * n],
                                       och[:, 0 : 2 * n])
                off += 2 * n

    nc.compile()
    return nc


def _eff_weights(weight, kernels):
    """Host-side constant folding: effective channel-mix matrices (fp64)."""
    w = weight.astype(np.float64)
    k = kernels.astype(np.float64)
    k00, k01, k02 = k[:, :, 0, 0], k[:, :, 0, 1], k[:, :, 0, 2]
    k10, k11, k12 = k[:, :, 1, 0], k[:, :, 1, 1], k[:, :, 1, 2]
    k20, k21, k22 = k[:, :, 2, 0], k[:, :, 2, 1], k[:, :, 2, 2]

    den_oo = k22 + k20 + k02 + k00 + EPS
    return dict(
        Wee=w * k11 / (k11 + EPS),
        Wf=w * k12 / (k12 + k10 + EPS), Wd=w * k10 / (k12 + k10 + EPS),
        Wh=w * k21 / (k21 + k01 + EPS), Wb=w * k01 / (k21 + k01 + EPS),
        Wi=w * k22 / den_oo, Wg=w * k20 / den_oo,
        Wc=w * k02 / den_oo, Wa=w * k00 / den_oo,
        # edge classes (host-applied): w'=95 column, h'=95 row, corner
        Ef=w * k12 / (k12 + EPS),
        Ei=w * k22 / (k22 + k02 + EPS), Ec=w * k02 / (k22 + k02 + EPS),
        Rh=w * k21 / (k21 + EPS),
        Ri=w * k22 / (k22 + k20 + EPS), Rg=w * k20 / (k22 + k20 + EPS),
        Ci=w * k22 / (k22 + EPS),
    )


def _lhsT_tiles(mats):
    """Pack the three [128,128] lhsT matmul tiles into a [128, 384] bf16 map.

    lhsT[k, m] = W[m, k]; K rows 0:64 = x channels, 64:128 = x<<1 channels.
    """
    Z = np.zeros((64, 64))

    def blk(wtl, wtr, wbl, wbr):  # quadrant weights [Mcols 0:64 | 64:128]
        top = np.concatenate([wtl.T, wtr.T], axis=1)
        bot = np.concatenate([wbl.T, wbr.T], axis=1)
        return np.concatenate([top, bot], axis=0)

    A = blk(mats["Wee"], mats["Wf"], Z, mats["Wd"])
    B1 = blk(mats["Wh"], mats["Wi"], Z, mats["Wg"])
    B2 = blk(mats["Wb"], mats["Wc"], Z, mats["Wa"])
    wm = np.concatenate([A, B1, B2], axis=1)      # [128, 384]
    return np.ascontiguousarray(wm).astype(BF16)


def _make_in_maps(input, weight, kernels, bias):
    mats = _eff_weights(weight, kernels)
    wm = _lhsT_tiles(mats)
    x = np.asarray(input, np.float32)
    in_maps = []
    for core in range(8):
        b, half = core // 2, core % 2
        slab = np.zeros((C, SLAB, 48), np.float32)
        if half == 0:
            slab[:, :, :] = x[b, :, 0:25, :]
        else:
            slab[:, 0:24, :] = x[b, :, 24:48, :]
        flat = slab.reshape(C, L)
        xd = np.zeros((128, LP), BF16)
        xd[0:64, 0:L] = flat.astype(BF16)
        xd[64:128, 0 : L - 1] = flat[:, 1:L].astype(BF16)
        in_maps.append({"x": xd, "wm": wm})
    return in_maps


def _postprocess(results, input, weight, kernels, bias):
    """Interleave parity planes, apply h'=95 / w'=95 edge classes, add bias."""
    mats = _eff_weights(weight, kernels)
    x = np.asarray(input, np.float64)
    out = np.empty((B, O, HO, WO), np.float32)

    for core in range(8):
        b, half = core // 2, core % 2
        r = np.asarray(results[core]["out"]).astype(np.float32)  # [128, 2304]
        p1 = np.concatenate([r[:, 0:512], r[:, 1024:1536], r[:, 2048:2176]],
                            axis=1)
        p2 = np.concatenate([r[:, 512:1024], r[:, 1536:2048], r[:, 2176:2304]],
                            axis=1)
        ee = p1[0:64].reshape(O, 24, 48)
        eo = p1[64:128].reshape(O, 24, 48)
        oe = p2[0:64].reshape(O, 24, 48)
        oo = p2[64:128].reshape(O, 24, 48)
        oh = out[b, :, half * 48 : (half + 1) * 48, :]
        oh[:, 0::2, 0::2] = ee
        oh[:, 0::2, 1::2] = eo
        oh[:, 1::2, 0::2] = oe
        oh[:, 1::2, 1::2] = oo

    # --- edge fixups (exact, on full input) ---
    xc = x[:, :, :, 47]                       # [B, C, 48] last input column
    xr = x[:, :, 47, :]                       # [B, C, 48] last input row
    # w'=95, even h'=2r: Ef @ x[:, :, r, 47]
    out[:, :, 0::2, 95] = np.einsum("oc,bcr->bor", mats["Ef"], xc)
    # w'=95, odd h'=2r+1, r<47: Ei @ x[r,47] + Ec @ x[r+1,47]
    out[:, :, 1:95:2, 95] = (np.einsum("oc,bcr->bor", mats["Ei"], xc[:, :, :47])
                             + np.einsum("oc,bcr->bor", mats["Ec"], xc[:, :, 1:]))
    # h'=95, even w'=2q: Rh @ x[47, q]
    out[:, :, 95, 0::2] = np.einsum("oc,bcq->boq", mats["Rh"], xr)
    # h'=95, odd w'=2q+1, q<47: Ri @ x[47,q] + Rg @ x[47,q+1]
    out[:, :, 95, 1:95:2] = (np.einsum("oc,bcq->boq", mats["Ri"], xr[:, :, :47])
                             + np.einsum("oc,bcq->boq", mats["Rg"], xr[:, :, 1:]))
    # corner (95, 95): Ci @ x[47, 47]
    out[:, :, 95, 95] = np.einsum("oc,bc->bo", mats["Ci"], x[:, :, 47, 47])

    out += np.asarray(bias, np.float32)[None, :, None, None]
    return out


def kernel(input, weight, kernels, bias):
    from concourse.bass_utils import run_bass_kernel_spmd

    input = np.asarray(input)
    weight = np.asarray(weight)
    kernels = np.asarray(kernels)
    bias = np.asarray(bias)

    if "nc" not in _prog_cache:
        _prog_cache["nc"] = _build_program()
    nc = _prog_cache["nc"]

    in_maps = _make_in_maps(input, weight, kernels, bias)
    res = run_bass_kernel_spmd(nc, in_maps, core_ids=list(range(8)))
    return _postprocess(res.results, input, weight, kernels, bias)


# revision 8
# speedup vs baseline: 1.2950x; 1.2950x over previous
"""Trainium2 Bass kernel for nn_NormConvTranspose2d (v2, minimal device program).

Math: the reference applies, per (out-channel o, in-channel c), a
ConvTranspose2d(stride=2, k=3, pad=1, outpad=1) to input channel c with
kernel K[o,c], divides by the same convT applied to an all-ones image
(+eps), multiplies by weight[o,c], sums over c, adds bias.

With stride 2 / k 3, each output-pixel parity class (h'=2r+a, w'=2q+b)
is a fixed 1-4 tap correlation of the input, and the norm denominator is
a per-(o,c) constant within each parity class (except at the h'=95 /
w'=95 edges).  y/norm therefore folds into effective channel-mixing
matrices W_tap = weight*ktap/denom computed on the host, and the module
becomes channel-mixing matmuls over (shifted) input.

Device program (per core) computes ONLY the interior of the four parity
planes with 9 matmuls; everything else (plane interleave, h'=95 / w'=95
edge columns, bias) is cheap host post-processing:

  T1 = [x ; x<<1elem]  (128 partitions, built host-side in DRAM)
  P1 = [[Wee,Wf],[0,Wd]]  @ T1          -> [ee | eo]   (1 matmul)
  P2 = [[Wh,Wi],[0,Wg]]   @ T1          -> [oe | oo]   (accumulating
     + [[Wb,Wc],[0,Wa]]   @ (T1 << 48)                  pair)

Sharding: 8 cores = 4 batches x 2 output-row halves, no communication.
Each core: 2 input DMAs (bf16), 9 matmuls over 3 column-chunks
{512,512,128}, DVE/ACT psum->sbuf bf16 copies, 3 output DMAs.  A couple
of zero warmup matmuls run in the input-DMA shadow to ramp the PE
p-state.
"""

import numpy as np
import ml_dtypes

BF16 = ml_dtypes.bfloat16
EPS = 1e-10
B, C, O, H, W = 4, 64, 64, 48, 48
HO = WO = 96
SLAB = 25          # input rows per core (24 + halo)
L = SLAB * 48      # 1200
LP = 1216          # padded free size of x tile
CHUNKS = [(0, 512), (512, 512), (1024, 128)]
N_WARMUP = 7
WM = 384           # weight-map columns, stored ahead of x in the fused tile
XW = WM + LP       # 1600 total columns
SPLIT = 992        # first input-DMA piece [0:SPLIT) covers wm + chunk-0 x

_prog_cache = {}


def _build_program():
    import concourse.mybir as mybir
    import concourse.tile as tile
    from concourse import bacc

    f32 = mybir.dt.float32
    bf16 = mybir.dt.bfloat16
    Ident = mybir.ActivationFunctionType.Identity

    nc = bacc.Bacc("TRN2", target_bir_lowering=False, debug=False, num_devices=8)
    xw_d = nc.dram_tensor("xw", [128, XW], bf16, kind="ExternalInput").ap()
    out_d = nc.dram_tensor("out", [128, 2304], bf16, kind="ExternalOutput").ap()

    with tile.TileContext(nc) as tc:
        with (
            tc.tile_pool(name="const", bufs=1) as cpool,
            tc.tile_pool(name="outp", bufs=3) as opool,
            tc.tile_pool(name="psum", bufs=3, space="PSUM") as ppool,
            tc.tile_pool(name="psumw", bufs=1, space="PSUM") as wpool,
        ):
            # warm the Scalar activation table before any data arrives
            warm = cpool.tile([64, 1], f32)
            nc.vector.memset(warm[:], 0.0)
            nc.scalar.activation(warm[:], warm[:], Ident, bias=0.0)

            # fused [wm | x-stacked] tile; piece 0 unblocks chunk-0 matmuls
            xw = cpool.tile([128, XW], bf16)
            nc.sync.dma_start(xw[:, 0:SPLIT], xw_d[:, 0:SPLIT])
            nc.scalar.dma_start(xw[:, SPLIT:XW], xw_d[:, SPLIT:XW])
            wm = xw[:, 0:WM]
            t1 = xw[:, WM:XW]

            # PE p-state warmup on zeros while the input DMA is in flight
            scr = cpool.tile([128, 512], bf16)
            nc.vector.memset(scr[:], 0.0)
            wps = wpool.tile([128, 512], f32, tag="wu")
            for _ in range(N_WARMUP):
                nc.tensor.matmul(wps[:], scr[:, 0:128], scr[:],
                                 start=True, stop=True)

            out_dmas = [nc.sync, nc.scalar, nc.sync]
            off = 0
            for ci, (fb, n) in enumerate(CHUNKS):
                p1 = ppool.tile([128, 512], f32, tag="p1")
                nc.tensor.matmul(p1[:, 0:n], wm[:, 0:128], t1[:, fb : fb + n],
                                 start=True, stop=True)
                p2 = ppool.tile([128, 512], f32, tag="p2")
                nc.tensor.matmul(p2[:, 0:n], wm[:, 128:256],
                                 t1[:, fb : fb + n], start=True, stop=False)
                nc.tensor.matmul(p2[:, 0:n], wm[:, 256:384],
                                 t1[:, fb + 48 : fb + 48 + n],
                                 start=False, stop=True)

                och = opool.tile([128, 1024], bf16)
                nc.vector.tensor_copy(och[:, 0:n], p1[:, 0:n])
                nc.scalar.copy(och[:, n : 2 * n], p2[:, 0:n])
                out_dmas[ci].dma_start(out_d[:, off : off + 2 * n],
                                       och[:, 0 : 2 * n])
                off += 2 * n

    nc.compile()
    return nc


def _eff_weights(weight, kernels):
    """Host-side constant folding: effective channel-mix matrices (fp64)."""
    w = weight.astype(np.float64)
    k = kernels.astype(np.float64)
    k00, k01, k02 = k[:, :, 0, 0], k[:, :, 0, 1], k[:, :, 0, 2]
    k10, k11, k12 = k[:, :, 1, 0], k[:, :, 1, 1], k[:, :, 1, 2]
    k20, k21, k22 = k[:, :, 2, 0], k[:, :, 2, 1], k[:, :, 2, 2]

    den_oo = k22 + k20 + k02 + k00 + EPS
    return dict(
        Wee=w * k11 / (k11 + EPS),
        Wf=w * k12 / (k12 + k10 + EPS), Wd=w * k10 / (k12 + k10 + EPS),
        Wh=w * k21 / (k21 + k01 + EPS), Wb=w * k01 / (k21 + k01 + EPS),
        Wi=w * k22 / den_oo, Wg=w * k20 / den_oo,
        Wc=w * k02 / den_oo, Wa=w * k00 / den_oo,
        # edge classes (host-applied): w'=95 column, h'=95 row, corner
        Ef=w * k12 / (k12 + EPS),
        Ei=w * k22 / (k22 + k02 + EPS), Ec=w * k02 / (k22 + k02 + EPS),
        Rh=w * k21 / (k21 + EPS),
        Ri=w * k22 / (k22 + k20 + EPS), Rg=w * k20 / (k22 + k20 + EPS),
        Ci=w * k22 / (k22 + EPS),
    )


def _lhsT_tiles(mats):
    """Pack the three [128,128] lhsT matmul tiles into a [128, 384] bf16 map.

    lhsT[k, m] = W[m, k]; K rows 0:64 = x channels, 64:128 = x<<1 channels.
    """
    Z = np.zeros((64, 64))

    def blk(wtl, wtr, wbl, wbr):  # quadrant weights [Mcols 0:64 | 64:128]
        top = np.concatenate([wtl.T, wtr.T], axis=1)
        bot = np.concatenate([wbl.T, wbr.T], axis=1)
        return np.concatenate([top, bot], axis=0)

    A = blk(mats["Wee"], mats["Wf"], Z, mats["Wd"])
    B1 = blk(mats["Wh"], mats["Wi"], Z, mats["Wg"])
    B2 = blk(mats["Wb"], mats["Wc"], Z, mats["Wa"])
    wm = np.concatenate([A, B1, B2], axis=1)      # [128, 384]
    return np.ascontiguousarray(wm).astype(BF16)


def _make_in_maps(input, weight, kernels, bias):
    mats = _eff_weights(weight, kernels)
    wm = _lhsT_tiles(mats)
    x = np.asarray(input, np.float32)
    in_maps = []
    for core in range(8):
        b, half = core // 2, core % 2
        slab = np.zeros((C, SLAB, 48), np.float32)
        if half == 0:
            slab[:, :, :] = x[b, :, 0:25, :]
        else:
            slab[:, 0:24, :] = x[b, :, 24:48, :]
        flat = slab.reshape(C, L)
        xwd = np.zeros((128, XW), BF16)
        xwd[:, 0:WM] = wm
        xwd[0:64, WM : WM + L] = flat.astype(BF16)
        xwd[64:128, WM : WM + L - 1] = flat[:, 1:L].astype(BF16)
        in_maps.append({"xw": xwd})
    return in_maps


def _postprocess(results, input, weight, kernels, bias):
    """Interleave parity planes, apply h'=95 / w'=95 edge classes, add bias."""
    mats = _eff_weights(weight, kernels)
    x = np.asarray(input, np.float64)
    out = np.empty((B, O, HO, WO), np.float32)

    for core in range(8):
        b, half = core // 2, core % 2
        r = np.asarray(results[core]["out"]).astype(np.float32)  # [128, 2304]
        p1 = np.concatenate([r[:, 0:512], r[:, 1024:1536], r[:, 2048:2176]],
                            axis=1)
        p2 = np.concatenate([r[:, 512:1024], r[:, 1536:2048], r[:, 2176:2304]],
                            axis=1)
        ee = p1[0:64].reshape(O, 24, 48)
        eo = p1[64:128].reshape(O, 24, 48)
        oe = p2[0:64].reshape(O, 24, 48)
        oo = p2[64:128].reshape(O, 24, 48)
        oh = out[b, :, half * 48 : (half + 1) * 48, :]
        oh[:, 0::2, 0::2] = ee
        oh[:, 0::2, 1::2] = eo
        oh[:, 1::2, 0::2] = oe
        oh[:, 1::2, 1::2] = oo

    # --- edge fixups (exact, on full input) ---
    xc = x[:, :, :, 47]                       # [B, C, 48] last input column
    xr = x[:, :, 47, :]                       # [B, C, 48] last input row
    # w'=95, even h'=2r: Ef @ x[:, :, r, 47]
    out[:, :, 0::2, 95] = np.einsum("oc,bcr->bor", mats["Ef"], xc)
    # w'=95, odd h'=2r+1, r<47: Ei @ x[r,47] + Ec @ x[r+1,47]
    out[:, :, 1:95:2, 95] = (np.einsum("oc,bcr->bor", mats["Ei"], xc[:, :, :47])
                             + np.einsum("oc,bcr->bor", mats["Ec"], xc[:, :, 1:]))
    # h'=95, even w'=2q: Rh @ x[47, q]
    out[:, :, 95, 0::2] = np.einsum("oc,bcq->boq", mats["Rh"], xr)
    # h'=95, odd w'=2q+1, q<47: Ri @ x[47,q] + Rg @ x[47,q+1]
    out[:, :, 95, 1:95:2] = (np.einsum("oc,bcq->boq", mats["Ri"], xr[:, :, :47])
                             + np.einsum("oc,bcq->boq", mats["Rg"], xr[:, :, 1:]))
    # corner (95, 95): Ci @ x[47, 47]
    out[:, :, 95, 95] = np.einsum("oc,bc->bo", mats["Ci"], x[:, :, 47, 47])

    out += np.asarray(bias, np.float32)[None, :, None, None]
    return out


def kernel(input, weight, kernels, bias):
    from concourse.bass_utils import run_bass_kernel_spmd

    input = np.asarray(input)
    weight = np.asarray(weight)
    kernels = np.asarray(kernels)
    bias = np.asarray(bias)

    if "nc" not in _prog_cache:
        _prog_cache["nc"] = _build_program()
    nc = _prog_cache["nc"]

    in_maps = _make_in_maps(input, weight, kernels, bias)
    res = run_bass_kernel_spmd(nc, in_maps, core_ids=list(range(8)))
    return _postprocess(res.results, input, weight, kernels, bias)
